# revision 34
# baseline (speedup 1.0000x reference)
"""CrossAttention kernel for 8 Trainium2 NeuronCores.

Problem (hardcoded shapes): B=4, N=1024, C=1024, E=1024, H=16, D=64.
  kv = x @ Wkv + bkv ; k, v = split(kv) ; q = query @ Wq + bq
  keys = [k; q] (2N), values = [v; v]
  out = softmax(q keys^T / sqrt(D)) @ values        -> [B, N, E]

Sharding: 8 cores = 4 batches x 2 head-groups (8 heads each).

Per-core design (PE/ACT/DVE all run ~70-100% through the attention span):
  - bf16 inputs/weights, host-pretiled to SBUF layouts; few large DMAs
    ordered so pair 0's q-projection chain loads first.
  - attention runs per head-pair; the q-as-keys half of the key range runs
    FIRST so exp starts as soon as q^T is projected (x may still be loading).
  - exp work is split: most tiles on the ACT engine (exact), ~1/4 on the DVE
    via a Schraudolph bit-trick (bitcast(round(s*A+B)) as bf16, ~1.8% rms),
    sized so ACT, PE and DVE busy times all land near 110us.
  - values are duplicated across the two key halves, so PV contracts over
    probs1+probs2 (one bf16 DVE add per tile) - half the PV matmul work.
  - PV orientation out[q, d]: stationary = summed probs [keys,128q] bf16,
    moving = v-tile [keys, 65] bf16 (65th col = ones -> softmax denominator).
    PSUM: scores triple-buffered (6 banks) + 1 rotating projection bank +
    1 PV bank; PV runs in qc-group passes (73-elem slots so no matmul output
    crosses a bank; start=True zeroes a whole bank so only the first
    accumulator of a bank "starts"), later passes ride the next pair's
    q-part through a background work pump that also spreads the next pair's
    projections into the PE's slack between score matmuls.
  - biases folded into the PSUM->SBUF copies on DVE; normalization =
    reciprocal of the denominator column + per-partition scalar multiplies
    split across DVE and ACT; output stored [N, EC] directly.
"""
import numpy as np

B, N, C, E, H = 4, 1024, 1024, 1024, 16
D = E // H            # 64
HPC = 8               # heads per core
EC = HPC * D          # 512 E-columns per core
NCORES = 8
CT = C // 128         # 8 contraction tiles
ST = N // 128         # 8 seq tiles
PAIRS = HPC // 2      # 4 head pairs
KB = N // 128         # 8 key blocks per key half

_compiled = None


def _build():
    import concourse.bass as bass
    import concourse.bacc as bacc
    import concourse.mybir as mybir
    import concourse.tile as tile
    import contextlib
    from collections import deque

    F32 = mybir.dt.float32
    F32R = mybir.dt.float32r
    BF16 = mybir.dt.bfloat16
    I16 = mybir.dt.int16
    SCH_A = 128 * 1.4426950408889634 * 0.125   # schraudolph slope (raw score)
    SCH_B = 127 * 128.0 - 7.5                  # bias + rms-centering offset
    EXP = mybir.ActivationFunctionType.Exp
    ADD = mybir.AluOpType.add
    MULT = mybir.AluOpType.mult

    nc = bacc.Bacc()
    xT_in = nc.declare_dram_parameter("xT", [128, CT, N], BF16, isOutput=False)
    qryT_in = nc.declare_dram_parameter("qryT", [128, CT, N], BF16, isOutput=False)
    wq_in = nc.declare_dram_parameter("wq", [128, PAIRS, CT, 128], BF16, isOutput=False)
    wk_in = nc.declare_dram_parameter("wk", [128, PAIRS, CT, 128], BF16, isOutput=False)
    wv_in = nc.declare_dram_parameter("wv", [128, PAIRS, CT, 128], BF16, isOutput=False)
    bq_in = nc.declare_dram_parameter("bqc", [128, PAIRS], F32, isOutput=False)
    bk_in = nc.declare_dram_parameter("bkc", [128, PAIRS], F32, isOutput=False)
    bv_in = nc.declare_dram_parameter("bvv", [128, EC], BF16, isOutput=False)
    out_o = nc.declare_dram_parameter("out_t", [N, EC], F32, isOutput=True)

    with tile.TileContext(nc) as tc, contextlib.ExitStack() as ctx:
        pers = ctx.enter_context(tc.tile_pool(name="pers", bufs=1))
        ekp = ctx.enter_context(tc.tile_pool(name="ekp", bufs=5))
        esp = ctx.enter_context(tc.tile_pool(name="esp", bufs=18))
        outp = ctx.enter_context(tc.tile_pool(name="outp", bufs=4))
        prj = ctx.enter_context(tc.tile_pool(name="prj", bufs=1, space="PSUM"))
        scp = ctx.enter_context(tc.tile_pool(name="scp", bufs=3, space="PSUM"))
        pvp = ctx.enter_context(tc.tile_pool(name="pvp", bufs=1, space="PSUM"))

        # ---- persistent SBUF ----
        xTs = pers.tile([128, CT, N], BF16, tag="xTs")
        qryTs = pers.tile([128, CT, N], BF16, tag="qryTs")
        wqs = pers.tile([128, PAIRS, CT, 128], BF16, tag="wqs")
        wks = pers.tile([128, PAIRS, CT, 128], BF16, tag="wks")
        wvs = pers.tile([128, PAIRS, CT, 128], BF16, tag="wvs")
        qTs = pers.tile([128, PAIRS, N], BF16, tag="qTs")
        kTs = pers.tile([128, PAIRS, N], BF16, tag="kTs")
        vvs = pers.tile([128, ST, HPC, 66], BF16, tag="vvs")
        bqr = pers.tile([128, PAIRS], F32, tag="bqr")
        bkr = pers.tile([128, PAIRS], F32, tag="bkr")
        bvr = pers.tile([128, EC], BF16, tag="bvr")
        # q-part probs for the current pair (overwritten each pair)
        eqs = pers.tile([128, KB, 2, N], BF16, tag="eqs")

        # ---- loads, priority order (DMA is serial): biases, then the
        # q-projection chain (wq pair0 -> qryT), then k (wk p0 -> xT), v,
        # then remaining pairs' weights.
        nc.sync.dma_start(out=wqs[:, 0], in_=wq_in[:, 0])
        for c2 in range(CT // 2):
            nc.sync.dma_start(out=qryTs[:, 2 * c2:2 * c2 + 2],
                              in_=qryT_in[:, 2 * c2:2 * c2 + 2])
        nc.sync.dma_start(out=bqr[:], in_=bq_in[:, :])
        nc.sync.dma_start(out=bkr[:], in_=bk_in[:, :])
        nc.sync.dma_start(out=wks[:, 0], in_=wk_in[:, 0])
        for c2 in range(CT // 2):
            nc.sync.dma_start(out=xTs[:, 2 * c2:2 * c2 + 2],
                              in_=xT_in[:, 2 * c2:2 * c2 + 2])
        nc.sync.dma_start(out=bvr[:], in_=bv_in[:, :])
        nc.sync.dma_start(out=wvs[:, 0], in_=wv_in[:, 0])
        for p in range(1, PAIRS):
            nc.sync.dma_start(out=wqs[:, p], in_=wq_in[:, p])
            nc.sync.dma_start(out=wks[:, p], in_=wk_in[:, p])
            nc.sync.dma_start(out=wvs[:, p], in_=wv_in[:, p])

        wrm = pers.tile([128, 512], BF16, tag="wrm")
        nc.vector.memset(wrm[:], 0.5)
        nc.vector.memset(vvs[:, :, :, 64:65], 1.0)
        # PE p-state warmup: the cost model ramps the PE clock from 0.65GHz to
        # 2.4GHz over ~3us of busy time measured from the first matmul; burn
        # that ramp on dummies while the DMAs land so the real projections run
        # at full clock.
        for _ in range(8):
            pw = prj.tile([128, 512], F32, tag="prj", name="pw")
            nc.tensor.matmul(pw[:], wrm[:, 0:128], wrm[:], start=True, stop=True)

        # ---- projection emitters (yield every couple of matmuls so they can
        # be pumped into the PE stream between attention steps) ----
        def gen_1_proj(p, wsb, src, dstT, brow):
            for half in range(2):
                hsl = slice(half * 512, (half + 1) * 512)
                pt = prj.tile([128, 512], F32, tag="prj")
                for ct in range(CT):
                    nc.tensor.matmul(pt[:], wsb[:, p, ct, :],
                                     src[:, ct, hsl],
                                     start=(ct == 0), stop=(ct == CT - 1))
                    yield 214
                nc.vector.tensor_scalar(out=dstT[:, p, hsl], in0=pt[:],
                                        scalar1=brow[:, p:p + 1],
                                        scalar2=None, op0=ADD)
                yield 0

        def gen_q_proj(p):
            yield from gen_1_proj(p, wqs, qryTs, qTs, bqr)

        def gen_k_proj(p):
            yield from gen_1_proj(p, wks, xTs, kTs, bkr)

        def gen_v_proj(p):
            for g in range(2):
                pt = prj.tile([128, 4, 128], F32, tag="prj")
                for ct in range(CT):
                    for si in range(4):
                        st = g * 4 + si
                        # start=True zeroes the WHOLE psum bank: only the
                        # first matmul of the bank starts; siblings accumulate
                        # onto the zeroed bank.
                        nc.tensor.matmul(pt[:, si, :],
                                         xTs[:, ct, st * 128:(st + 1) * 128],
                                         wvs[:, p, ct, :],
                                         start=(ct == 0 and si == 0),
                                         stop=(ct == CT - 1),
                                         skip_group_check=True)
                    yield 214
                for si in range(4):
                    st = g * 4 + si
                    nc.vector.tensor_add(
                        out=vvs[:, st, 2 * p:2 * p + 2, 0:64],
                        in0=pt[:, si, :].rearrange("q (h d) -> q h d", h=2),
                        in1=bvr[:, p * 128:(p + 1) * 128].rearrange(
                            "q (h d) -> q h d", h=2))
                yield 0

        bg = deque()

        def pump(budget_ns):
            # pull background PE work until ~budget_ns of matmul time emitted
            while budget_ns > 0 and bg:
                try:
                    budget_ns -= next(bg[0])
                except StopIteration:
                    bg.popleft()

        # PV accumulators: 16 accs (hi*8+qc) packed 7/7/2 into 3 banks.
        def acc_of(tiles, j):
            if j < 7:
                return tiles[0], j
            if j < 14:
                return tiles[1], j - 7
            return tiles[2], j - 14

        HI = ((0, slice(0, 64)), (1, slice(64, 128)))

        def emit_sc(p, src, kb, rows):
            ksl = slice(kb * 128, (kb + 1) * 128)
            sct = scp.tile([128, N], F32, tag="sc")
            nc.tensor.matmul(sct[:, 0:512], src[rows, p, ksl],
                             qTs[rows, p, 0:512])
            nc.tensor.matmul(sct[:, 512:1024], src[rows, p, ksl],
                             qTs[rows, p, 512:1024])
            return sct

        def emit_sch(sct, e):
            # schraudolph exp on DVE: bitcast(round(s*A+B)) as bf16
            nc.vector.tensor_scalar(out=e.bitcast(I16), in0=sct[:],
                                    scalar1=SCH_A, scalar2=SCH_B,
                                    op0=MULT, op1=ADD)

        def emit_pv2(p, tiles, quads, kb, es2, start, stop):
            # quads: list of qc; accs i = hi*len(quads) + idx; tiles hold
            # 7 + overflow slots
            nq = len(quads)
            for hi in range(2):
                for qi, qc in enumerate(quads):
                    i = hi * nq + qi
                    t, jj = (tiles[0], i) if i < 7 else (tiles[1], i - 7)
                    nc.tensor.matmul(t[:, jj, 0:65],
                                     es2[hi][:, qc * 128:(qc + 1) * 128],
                                     vvs[:, kb, 2 * p + hi, 0:65],
                                     start=(start and jj == 0),
                                     stop=stop, skip_group_check=True)

        # ---- head: only pair 0's q projection runs eagerly (its DMA chain
        # loads first); k/v of pair 0 and everything for later pairs pump
        # through the PE's slack during attention steps.
        pt1 = prj.tile([128, 512], F32, tag="prj", name="qp1")
        pt2t = scp.tile([128, 512], F32, tag="sc", name="qp2")
        pt2 = pt2t[:]
        for ct in range(CT):
            nc.tensor.matmul(pt1[:], wqs[:, 0, ct, :], qryTs[:, ct, 0:512],
                             start=(ct == 0), stop=(ct == CT - 1))
            nc.tensor.matmul(pt2, wqs[:, 0, ct, :], qryTs[:, ct, 512:1024],
                             start=(ct == 0), stop=(ct == CT - 1))
        nc.vector.tensor_scalar(out=qTs[:, 0, 0:512], in0=pt1[:],
                                scalar1=bqr[:, 0:1], scalar2=None, op0=ADD)
        nc.vector.tensor_scalar(out=qTs[:, 0, 512:1024], in0=pt2,
                                scalar1=bqr[:, 0:1], scalar2=None, op0=ADD)
        bg.append(gen_k_proj(0))
        bg.append(gen_v_proj(0))
        for np_ in range(1, PAIRS):
            bg.append(gen_q_proj(np_))
            bg.append(gen_k_proj(np_))
            bg.append(gen_v_proj(np_))

        for p in range(PAIRS):
            # ---- q-as-keys half first: needs only qTs ----
            for kb in range(KB):
                for hi, rows in HI:
                    sct = emit_sc(p, qTs, kb, rows)
                    if hi == 1 and 1 <= kb <= 6:
                        emit_sch(sct, eqs[:, kb, hi, :])
                    else:
                        nc.scalar.activation(out=eqs[:, kb, hi, :],
                                             in_=sct[:], func=EXP,
                                             scale=0.125)
                pump(0 if kb >= KB - 1 else 1300)
            # ---- k half. PV runs in qc-group passes sharing one PSUM bank:
            # pairs 0-2: pass {0,1} inline, passes {2,3},{4,5},{6,7} deferred
            # into the next pair's q-part via the background pump.
            # Last pair: two 8-acc passes (borrowing the freed proj bank),
            # second pass in the tail.
            last = p == PAIRS - 1
            if last:
                prjL = prj.tile([128, 7, 73], F32, tag="prj", name="prjL")
                pv1 = (pvp.tile([128, 7, 73], F32, tag="pv", name="pvL"),
                       prjL)
                quads1 = [0, 1, 2, 3]

                def emit_pv679(kb, es2, stop=False):
                    # qc 4..6 accumulate inline in prjL slots 1..6 (slot 0
                    # belongs to pass 1's 8th acc, whose kb0 matmul zeroes
                    # the bank first)
                    for hi in range(2):
                        for qi in range(3):
                            qc = 4 + qi
                            slot = 1 + hi * 3 + qi
                            nc.tensor.matmul(
                                prjL[:, slot, 0:65],
                                es2[hi][:, qc * 128:(qc + 1) * 128],
                                vvs[:, kb, 2 * p + hi, 0:65],
                                start=False, stop=stop,
                                skip_group_check=True)
            else:
                pv1 = (pvp.tile([128, 4, 73], F32, tag="pv", name="pv1"),
                       None)
                quads1 = [0, 1]
            es_all = []
            pend = None
            for kb in range(KB):
                es_hi = []
                for hi, rows in HI:
                    sct = emit_sc(p, kTs, kb, rows)
                    e = ekp.tile([128, N], BF16, tag="ek")
                    if hi == 1 and kb in (2, 4, 6):
                        emit_sch(sct, e[:])
                    else:
                        nc.scalar.activation(out=e[:], in_=sct[:], func=EXP,
                                             scale=0.125)
                    es_hi.append(e)
                if kb < KB - 1:
                    cur = []
                    for hi in range(2):
                        est = esp.tile([128, N], BF16, tag="es")
                        nc.vector.tensor_add(out=est[:], in0=es_hi[hi],
                                             in1=eqs[:, kb, hi, :])
                        cur.append(est)
                else:
                    cur = es_hi  # last kb: PV runs on both halves, no add
                es_all.append(cur)
                if pend is not None:
                    emit_pv2(p, pv1, quads1, kb - 1, pend,
                             start=(kb == 1), stop=False)
                    if last:
                        emit_pv679(kb - 1, pend)
                pend = cur
                if 0 < kb < KB - 1:
                    pump(550 if last else 800)
                elif kb == 0:
                    pump(0)
            eq7 = [eqs[:, KB - 1, 0, :], eqs[:, KB - 1, 1, :]]
            emit_pv2(p, pv1, quads1, KB - 1, pend, start=False, stop=False)
            emit_pv2(p, pv1, quads1, KB - 1, eq7, start=False, stop=True)
            if last:
                emit_pv679(KB - 1, pend)
                emit_pv679(KB - 1, eq7, stop=True)

            ost = outp.tile([128, ST, 128], F32, tag="osb")

            def norm_store(tiles, quads, ost=ost, p=p, on_act=False):
                # reciprocal + per-acc scalar multiply + one store
                nq = len(quads)
                r0 = outp.tile([128, 7, 1], F32, tag="rA", name="r0")
                nc.vector.reciprocal(out=r0[:, 0:min(2 * nq, 7), :],
                                     in_=tiles[0][:, 0:min(2 * nq, 7), 64:65])
                if 2 * nq > 7:
                    r1 = outp.tile([128, 1, 1], F32, tag="rB", name="r1")
                    nc.vector.reciprocal(out=r1[:],
                                         in_=tiles[1][:, 0:1, 64:65])
                for hi in range(2):
                    for qi, qc in enumerate(quads):
                        i = hi * nq + qi
                        t, jj = (tiles[0], i) if i < 7 else (tiles[1], i - 7)
                        r_ap = r0[:, i, :] if i < 7 else r1[:, i - 7, :]
                        dst = ost[:, qc, hi * 64:hi * 64 + 64]
                        if on_act and hi == 1:  # split muls across ACT/DVE
                            nc.scalar.mul(dst, t[:, jj, 0:64], r_ap)
                        else:
                            nc.vector.tensor_scalar(
                                out=dst, in0=t[:, jj, 0:64], scalar1=r_ap,
                                scalar2=None, op0=MULT)
                lo, hi_r = quads[0] * 128, (quads[-1] + 1) * 128
                nc.sync.dma_start(
                    out=out_o[lo:hi_r, p * 128:(p + 1) * 128].rearrange(
                        "(qc qi) c -> qi qc c", qi=128),
                    in_=ost[:, quads[0]:quads[-1] + 1, :])

            if last:
                norm_store(pv1, quads1, on_act=False)
                # qc 4..6 from prjL slots 1..6
                r2 = outp.tile([128, 6, 1], F32, tag="rA", name="r2")
                nc.vector.reciprocal(out=r2[:], in_=prjL[:, 1:7, 64:65])
                for hi in range(2):
                    for qi in range(3):
                        qc = 4 + qi
                        slot = 1 + hi * 3 + qi
                        dst = ost[:, qc, hi * 64:hi * 64 + 64]
                        if hi == 1:
                            nc.scalar.mul(dst, prjL[:, slot, 0:64],
                                          r2[:, slot - 1, :])
                        else:
                            nc.vector.tensor_scalar(
                                out=dst, in0=prjL[:, slot, 0:64],
                                scalar1=r2[:, slot - 1, :], scalar2=None,
                                op0=MULT)
                # qc 7: re-accumulate from the kept prob tiles in the pv bank
                pv3 = pvp.tile([128, 2, 73], F32, tag="pv", name="pv3")
                q7 = slice(7 * 128, 8 * 128)
                for kb in range(KB):
                    for hi in range(2):
                        nc.tensor.matmul(pv3[:, hi, 0:65],
                                         es_all[kb][hi][:, q7],
                                         vvs[:, kb, 2 * p + hi, 0:65],
                                         start=(kb == 0 and hi == 0),
                                         stop=False, skip_group_check=True)
                for hi in range(2):
                    nc.tensor.matmul(pv3[:, hi, 0:65], eq7[hi][:, q7],
                                     vvs[:, KB - 1, 2 * p + hi, 0:65],
                                     start=False, stop=True,
                                     skip_group_check=True)
                r3 = outp.tile([128, 2, 1], F32, tag="rB", name="r3")
                nc.vector.reciprocal(out=r3[:], in_=pv3[:, :, 64:65])
                for hi in range(2):
                    dst = ost[:, 7, hi * 64:hi * 64 + 64]
                    if hi == 1:
                        nc.scalar.mul(dst, pv3[:, hi, 0:64], r3[:, hi, :])
                    else:
                        nc.vector.tensor_scalar(
                            out=dst, in0=pv3[:, hi, 0:64],
                            scalar1=r3[:, hi, :], scalar2=None, op0=MULT)
                nc.sync.dma_start(
                    out=out_o[512:1024, p * 128:(p + 1) * 128].rearrange(
                        "(qc qi) c -> qi qc c", qi=128),
                    in_=ost[:, 4:8, :])
            else:
                norm_store(pv1, quads1, on_act=False)

                def gen_deferred(p=p, es_all=es_all, eq7=eq7, ost=ost):
                    for quads in ([2, 3], [4, 5], [6, 7]):
                        pvt = (pvp.tile([128, 4, 73], F32, tag="pv",
                                        name="pvd"), None)
                        for kb in range(KB):
                            emit_pv2(p, pvt, quads, kb, es_all[kb],
                                     start=(kb == 0), stop=False)
                            yield 120
                        emit_pv2(p, pvt, quads, KB - 1, eq7,
                                 start=False, stop=True)
                        yield 120
                        norm_store(pvt, quads, ost=ost, p=p, on_act=True)
                        yield 0
                bg.appendleft(gen_deferred())

    nc.finalize()
    return nc


def _get_compiled():
    global _compiled
    if _compiled is None:
        _compiled = _build()
    return _compiled


def kernel(x, query, Wkv, bkv, Wq, bq):
    import ml_dtypes
    from concourse.bass_utils import run_bass_kernel_spmd

    bf16 = ml_dtypes.bfloat16
    x = np.asarray(x, dtype=np.float32)
    query = np.asarray(query, dtype=np.float32)
    Wkv = np.asarray(Wkv, dtype=np.float32)
    bkv = np.asarray(bkv, dtype=np.float32)
    Wq = np.asarray(Wq, dtype=np.float32)
    bq = np.asarray(bq, dtype=np.float32)

    def tile_T(a):  # [N, C] -> [128, CT, N] (a.T tiled over contraction)
        return np.ascontiguousarray(
            a.T.reshape(CT, 128, N).transpose(1, 0, 2)).astype(bf16)

    def tile_w(w):  # [C, EC] -> [128, PAIRS, CT, 128]
        return np.ascontiguousarray(
            w.reshape(CT, 128, PAIRS, 128).transpose(1, 2, 0, 3)).astype(bf16)

    in_maps = []
    for core in range(NCORES):
        b, hg = core // 2, core % 2
        ecs = slice(hg * EC, (hg + 1) * EC)
        bv = bkv[E + hg * EC:E + (hg + 1) * EC]
        in_maps.append({
            "xT": tile_T(x[b]),
            "qryT": tile_T(query[b]),
            "wq": tile_w(Wq[:, ecs]),
            "wk": tile_w(Wkv[:, hg * EC:(hg + 1) * EC]),
            "wv": tile_w(Wkv[:, E + hg * EC:E + (hg + 1) * EC]),
            "bqc": np.ascontiguousarray(bq[ecs].reshape(PAIRS, 128).T),
            "bkc": np.ascontiguousarray(
                bkv[hg * EC:(hg + 1) * EC].reshape(PAIRS, 128).T),
            "bvv": np.ascontiguousarray(
                np.tile(bv[None, :], (128, 1)).astype(bf16)),
        })

    nc = _get_compiled()
    out = np.empty((B, N, E), np.float32)
    last_err = None
    for attempt in range(4):
        try:
            res = run_bass_kernel_spmd(nc, in_maps, list(range(NCORES)))
        except Exception as ex:  # transient NRT_EXEC_UNIT_UNRECOVERABLE etc.
            last_err = ex
            continue
        for core in range(NCORES):
            b, hg = core // 2, core % 2
            out[b, :, hg * EC:(hg + 1) * EC] = res.results[core]["out_t"]
        if np.isfinite(out).all():  # guard against transient device flakes
            return out
        last_err = RuntimeError("non-finite values in kernel output")
    raise last_err


# revision 35
# speedup vs baseline: 1.0301x; 1.0301x over previous
"""CrossAttention kernel for 8 Trainium2 NeuronCores.

Problem (hardcoded shapes): B=4, N=1024, C=1024, E=1024, H=16, D=64.
  kv = x @ Wkv + bkv ; k, v = split(kv) ; q = query @ Wq + bq
  keys = [k; q] (2N), values = [v; v]
  out = softmax(q keys^T / sqrt(D)) @ values        -> [B, N, E]

Sharding: 8 cores = 4 batches x 2 head-groups (8 heads each).

Per-core design (PE/ACT/DVE all run ~70-100% through the attention span):
  - bf16 inputs/weights, host-pretiled to SBUF layouts; few large DMAs
    ordered so pair 0's q-projection chain loads first.
  - attention runs per head-pair; the q-as-keys half of the key range runs
    FIRST so exp starts as soon as q^T is projected (x may still be loading).
  - exp work is split: most tiles on the ACT engine (exact), ~1/4 on the DVE
    via a Schraudolph bit-trick (bitcast(round(s*A+B)) as bf16, ~1.8% rms),
    sized so ACT, PE and DVE busy times all land near 110us.
  - values are duplicated across the two key halves, so PV contracts over
    probs1+probs2 (one bf16 DVE add per tile) - half the PV matmul work.
  - PV orientation out[q, d]: stationary = summed probs [keys,128q] bf16,
    moving = v-tile [keys, 65] bf16 (65th col = ones -> softmax denominator).
    PSUM: scores triple-buffered (6 banks) + 1 rotating projection bank +
    1 PV bank; PV runs in qc-group passes (73-elem slots so no matmul output
    crosses a bank; start=True zeroes a whole bank so only the first
    accumulator of a bank "starts"), later passes ride the next pair's
    q-part through a background work pump that also spreads the next pair's
    projections into the PE's slack between score matmuls.
  - biases folded into the PSUM->SBUF copies on DVE; normalization =
    reciprocal of the denominator column + per-partition scalar multiplies
    split across DVE and ACT; output stored [N, EC] directly.
"""
import numpy as np

B, N, C, E, H = 4, 1024, 1024, 1024, 16
D = E // H            # 64
HPC = 8               # heads per core
EC = HPC * D          # 512 E-columns per core
NCORES = 8
CT = C // 128         # 8 contraction tiles
ST = N // 128         # 8 seq tiles
PAIRS = HPC // 2      # 4 head pairs
KB = N // 128         # 8 key blocks per key half

_compiled = None


def _build():
    import concourse.bass as bass
    import concourse.bacc as bacc
    import concourse.mybir as mybir
    import concourse.tile as tile
    import contextlib
    from collections import deque

    F32 = mybir.dt.float32
    F32R = mybir.dt.float32r
    BF16 = mybir.dt.bfloat16
    I16 = mybir.dt.int16
    SCH_A = 128 * 1.4426950408889634 * 0.125   # schraudolph slope (raw score)
    SCH_B = 127 * 128.0 - 7.5                  # bias + rms-centering offset
    EXP = mybir.ActivationFunctionType.Exp
    ADD = mybir.AluOpType.add
    MULT = mybir.AluOpType.mult

    nc = bacc.Bacc()
    xT_in = nc.declare_dram_parameter("xT", [128, CT, N], BF16, isOutput=False)
    qryT_in = nc.declare_dram_parameter("qryT", [128, CT, N], BF16, isOutput=False)
    wq_in = nc.declare_dram_parameter("wq", [128, PAIRS, CT, 128], BF16, isOutput=False)
    wk_in = nc.declare_dram_parameter("wk", [128, PAIRS, CT, 128], BF16, isOutput=False)
    wv_in = nc.declare_dram_parameter("wv", [128, PAIRS, CT, 128], BF16, isOutput=False)
    bq_in = nc.declare_dram_parameter("bqc", [128, PAIRS], F32, isOutput=False)
    bk_in = nc.declare_dram_parameter("bkc", [128, PAIRS], F32, isOutput=False)
    bv_in = nc.declare_dram_parameter("bvv", [128, EC], BF16, isOutput=False)
    out_o = nc.declare_dram_parameter("out_t", [N, EC], F32, isOutput=True)

    with tile.TileContext(nc) as tc, contextlib.ExitStack() as ctx:
        pers = ctx.enter_context(tc.tile_pool(name="pers", bufs=1))
        ekp = ctx.enter_context(tc.tile_pool(name="ekp", bufs=5))
        esp = ctx.enter_context(tc.tile_pool(name="esp", bufs=18))
        outp = ctx.enter_context(tc.tile_pool(name="outp", bufs=4))
        prj = ctx.enter_context(tc.tile_pool(name="prj", bufs=1, space="PSUM"))
        scp = ctx.enter_context(tc.tile_pool(name="scp", bufs=3, space="PSUM"))
        pvp = ctx.enter_context(tc.tile_pool(name="pvp", bufs=1, space="PSUM"))

        # ---- persistent SBUF ----
        xTs = pers.tile([128, CT, N], BF16, tag="xTs")
        qryTs = pers.tile([128, CT, N], BF16, tag="qryTs")
        wqs = pers.tile([128, PAIRS, CT, 128], BF16, tag="wqs")
        wks = pers.tile([128, PAIRS, CT, 128], BF16, tag="wks")
        wvs = pers.tile([128, PAIRS, CT, 128], BF16, tag="wvs")
        qTs = pers.tile([128, PAIRS, N], BF16, tag="qTs")
        kTs = pers.tile([128, PAIRS, N], BF16, tag="kTs")
        vvs = pers.tile([128, ST, HPC, 66], BF16, tag="vvs")
        bqr = pers.tile([128, PAIRS], F32, tag="bqr")
        bkr = pers.tile([128, PAIRS], F32, tag="bkr")
        bvr = pers.tile([128, EC], BF16, tag="bvr")
        # q-part probs for the current pair (overwritten each pair)
        eqs = pers.tile([128, KB, 2, N], BF16, tag="eqs")

        # ---- loads, priority order (DMA is serial): biases, then the
        # q-projection chain (wq pair0 -> qryT), then k (wk p0 -> xT), v,
        # then remaining pairs' weights.
        nc.sync.dma_start(out=wqs[:, 0], in_=wq_in[:, 0])
        for c2 in range(CT // 2):
            nc.sync.dma_start(out=qryTs[:, 2 * c2:2 * c2 + 2],
                              in_=qryT_in[:, 2 * c2:2 * c2 + 2])
        nc.sync.dma_start(out=bqr[:], in_=bq_in[:, :])
        nc.sync.dma_start(out=bkr[:], in_=bk_in[:, :])
        nc.sync.dma_start(out=wks[:, 0], in_=wk_in[:, 0])
        for c2 in range(CT // 2):
            nc.sync.dma_start(out=xTs[:, 2 * c2:2 * c2 + 2],
                              in_=xT_in[:, 2 * c2:2 * c2 + 2])
        nc.sync.dma_start(out=bvr[:], in_=bv_in[:, :])
        nc.sync.dma_start(out=wvs[:, 0], in_=wv_in[:, 0])
        for p in range(1, PAIRS):
            nc.sync.dma_start(out=wqs[:, p], in_=wq_in[:, p])
            nc.sync.dma_start(out=wks[:, p], in_=wk_in[:, p])
            nc.sync.dma_start(out=wvs[:, p], in_=wv_in[:, p])

        wrm = pers.tile([128, 512], BF16, tag="wrm")
        nc.vector.memset(wrm[:], 0.5)
        nc.vector.memset(vvs[:, :, :, 64:65], 1.0)
        # PE p-state warmup: the cost model ramps the PE clock from 0.65GHz to
        # 2.4GHz over ~3us of busy time measured from the first matmul; burn
        # that ramp on dummies while the DMAs land so the real projections run
        # at full clock.
        for _ in range(8):
            pw = prj.tile([128, 512], F32, tag="prj", name="pw")
            nc.tensor.matmul(pw[:], wrm[:, 0:128], wrm[:], start=True, stop=True)

        # ---- projection emitters (yield every couple of matmuls so they can
        # be pumped into the PE stream between attention steps) ----
        def gen_1_proj(p, wsb, src, dstT, brow):
            for half in range(2):
                hsl = slice(half * 512, (half + 1) * 512)
                pt = prj.tile([128, 512], F32, tag="prj")
                for ct in range(CT):
                    nc.tensor.matmul(pt[:], wsb[:, p, ct, :],
                                     src[:, ct, hsl],
                                     start=(ct == 0), stop=(ct == CT - 1))
                    yield 214
                nc.vector.tensor_scalar(out=dstT[:, p, hsl], in0=pt[:],
                                        scalar1=brow[:, p:p + 1],
                                        scalar2=None, op0=ADD)
                yield 0

        def gen_q_proj(p):
            yield from gen_1_proj(p, wqs, qryTs, qTs, bqr)

        def gen_k_proj(p):
            yield from gen_1_proj(p, wks, xTs, kTs, bkr)

        def gen_v_proj(p):
            for g in range(2):
                pt = prj.tile([128, 4, 128], F32, tag="prj")
                for ct in range(CT):
                    for si in range(4):
                        st = g * 4 + si
                        # start=True zeroes the WHOLE psum bank: only the
                        # first matmul of the bank starts; siblings accumulate
                        # onto the zeroed bank.
                        nc.tensor.matmul(pt[:, si, :],
                                         xTs[:, ct, st * 128:(st + 1) * 128],
                                         wvs[:, p, ct, :],
                                         start=(ct == 0 and si == 0),
                                         stop=(ct == CT - 1),
                                         skip_group_check=True)
                    yield 214
                for si in range(4):
                    st = g * 4 + si
                    nc.vector.tensor_add(
                        out=vvs[:, st, 2 * p:2 * p + 2, 0:64],
                        in0=pt[:, si, :].rearrange("q (h d) -> q h d", h=2),
                        in1=bvr[:, p * 128:(p + 1) * 128].rearrange(
                            "q (h d) -> q h d", h=2))
                yield 0

        bg = deque()

        def pump(budget_ns):
            # pull background PE work until ~budget_ns of matmul time emitted
            while budget_ns > 0 and bg:
                try:
                    budget_ns -= next(bg[0])
                except StopIteration:
                    bg.popleft()

        # PV accumulators: 16 accs (hi*8+qc) packed 7/7/2 into 3 banks.
        def acc_of(tiles, j):
            if j < 7:
                return tiles[0], j
            if j < 14:
                return tiles[1], j - 7
            return tiles[2], j - 14

        HI = ((0, slice(0, 64)), (1, slice(64, 128)))

        def emit_sc(p, src, kb, rows):
            ksl = slice(kb * 128, (kb + 1) * 128)
            sct = scp.tile([128, N], F32, tag="sc")
            nc.tensor.matmul(sct[:, 0:512], src[rows, p, ksl],
                             qTs[rows, p, 0:512])
            nc.tensor.matmul(sct[:, 512:1024], src[rows, p, ksl],
                             qTs[rows, p, 512:1024])
            return sct

        def emit_sch(sct, e):
            # schraudolph exp on DVE: bitcast(round(s*A+B)) as bf16
            nc.vector.tensor_scalar(out=e.bitcast(I16), in0=sct[:],
                                    scalar1=SCH_A, scalar2=SCH_B,
                                    op0=MULT, op1=ADD)

        def emit_pv2(p, tiles, quads, kb, es2, start, stop):
            # quads: list of qc; accs i = hi*len(quads) + idx; tiles hold
            # 7 + overflow slots
            nq = len(quads)
            for hi in range(2):
                for qi, qc in enumerate(quads):
                    i = hi * nq + qi
                    t, jj = (tiles[0], i) if i < 7 else (tiles[1], i - 7)
                    nc.tensor.matmul(t[:, jj, 0:65],
                                     es2[hi][:, qc * 128:(qc + 1) * 128],
                                     vvs[:, kb, 2 * p + hi, 0:65],
                                     start=(start and jj == 0),
                                     stop=stop, skip_group_check=True)

        # ---- head: only pair 0's q projection runs eagerly (its DMA chain
        # loads first); k/v of pair 0 and everything for later pairs pump
        # through the PE's slack during attention steps.
        pt1 = prj.tile([128, 512], F32, tag="prj", name="qp1")
        pt2t = scp.tile([128, 512], F32, tag="sc", name="qp2")
        pt2 = pt2t[:]
        for ct in range(CT):
            nc.tensor.matmul(pt1[:], wqs[:, 0, ct, :], qryTs[:, ct, 0:512],
                             start=(ct == 0), stop=(ct == CT - 1))
            nc.tensor.matmul(pt2, wqs[:, 0, ct, :], qryTs[:, ct, 512:1024],
                             start=(ct == 0), stop=(ct == CT - 1))
        nc.vector.tensor_scalar(out=qTs[:, 0, 0:512], in0=pt1[:],
                                scalar1=bqr[:, 0:1], scalar2=None, op0=ADD)
        nc.vector.tensor_scalar(out=qTs[:, 0, 512:1024], in0=pt2,
                                scalar1=bqr[:, 0:1], scalar2=None, op0=ADD)
        bg.append(gen_k_proj(0))
        bg.append(gen_v_proj(0))
        for np_ in range(1, PAIRS):
            bg.append(gen_q_proj(np_))
            bg.append(gen_k_proj(np_))
            bg.append(gen_v_proj(np_))

        for p in range(PAIRS):
            # ---- q-as-keys half first: needs only qTs ----
            for kb in range(KB):
                for hi, rows in HI:
                    sct = emit_sc(p, qTs, kb, rows)
                    if hi == 1 and 1 <= kb <= 6:
                        emit_sch(sct, eqs[:, kb, hi, :])
                    else:
                        nc.scalar.activation(out=eqs[:, kb, hi, :],
                                             in_=sct[:], func=EXP,
                                             scale=0.125)
                pump(0 if kb >= KB - 1 else 1200)
            # ---- k half. PV runs in qc-group passes sharing one PSUM bank:
            # pairs 0-2: pass {0,1} inline, passes {2,3},{4,5},{6,7} deferred
            # into the next pair's q-part via the background pump.
            # Last pair: two 8-acc passes (borrowing the freed proj bank),
            # second pass in the tail.
            last = p == PAIRS - 1
            if last:
                prjL = prj.tile([128, 7, 73], F32, tag="prj", name="prjL")
                pv1 = (pvp.tile([128, 7, 73], F32, tag="pv", name="pvL"),
                       prjL)
                quads1 = [0, 1, 2, 3]

                def emit_pv679(kb, es2, stop=False):
                    # qc 4..6 accumulate inline in prjL slots 1..6 (slot 0
                    # belongs to pass 1's 8th acc, whose kb0 matmul zeroes
                    # the bank first)
                    for hi in range(2):
                        for qi in range(3):
                            qc = 4 + qi
                            slot = 1 + hi * 3 + qi
                            nc.tensor.matmul(
                                prjL[:, slot, 0:65],
                                es2[hi][:, qc * 128:(qc + 1) * 128],
                                vvs[:, kb, 2 * p + hi, 0:65],
                                start=False, stop=stop,
                                skip_group_check=True)
            else:
                pv1 = (pvp.tile([128, 4, 73], F32, tag="pv", name="pv1"),
                       None)
                quads1 = [0, 1]
            es_all = []
            pend = None
            for kb in range(KB):
                es_hi = []
                for hi, rows in HI:
                    sct = emit_sc(p, kTs, kb, rows)
                    e = ekp.tile([128, N], BF16, tag="ek")
                    if hi == 1 and kb in (2, 4, 6):
                        emit_sch(sct, e[:])
                    else:
                        nc.scalar.activation(out=e[:], in_=sct[:], func=EXP,
                                             scale=0.125)
                    es_hi.append(e)
                if kb < KB - 1:
                    cur = []
                    for hi in range(2):
                        est = esp.tile([128, N], BF16, tag="es")
                        nc.vector.tensor_add(out=est[:], in0=es_hi[hi],
                                             in1=eqs[:, kb, hi, :])
                        cur.append(est)
                else:
                    cur = es_hi  # last kb: PV runs on both halves, no add
                es_all.append(cur)
                if pend is not None:
                    emit_pv2(p, pv1, quads1, kb - 1, pend,
                             start=(kb == 1), stop=False)
                    if last:
                        emit_pv679(kb - 1, pend)
                pend = cur
                if 0 < kb < KB - 1:
                    pump(550 if last else 850)
                elif kb == 0:
                    pump(0)
            eq7 = [eqs[:, KB - 1, 0, :], eqs[:, KB - 1, 1, :]]
            emit_pv2(p, pv1, quads1, KB - 1, pend, start=False, stop=False)
            emit_pv2(p, pv1, quads1, KB - 1, eq7, start=False, stop=True)
            if last:
                emit_pv679(KB - 1, pend)
                emit_pv679(KB - 1, eq7, stop=True)

            ost = outp.tile([128, ST, 128], F32, tag="osb")

            def norm_store(tiles, quads, ost=ost, p=p, on_act=False):
                # reciprocal + per-acc scalar multiply + one store
                nq = len(quads)
                r0 = outp.tile([128, 7, 1], F32, tag="rA", name="r0")
                nc.vector.reciprocal(out=r0[:, 0:min(2 * nq, 7), :],
                                     in_=tiles[0][:, 0:min(2 * nq, 7), 64:65])
                if 2 * nq > 7:
                    r1 = outp.tile([128, 1, 1], F32, tag="rB", name="r1")
                    nc.vector.reciprocal(out=r1[:],
                                         in_=tiles[1][:, 0:1, 64:65])
                for hi in range(2):
                    for qi, qc in enumerate(quads):
                        i = hi * nq + qi
                        t, jj = (tiles[0], i) if i < 7 else (tiles[1], i - 7)
                        r_ap = r0[:, i, :] if i < 7 else r1[:, i - 7, :]
                        dst = ost[:, qc, hi * 64:hi * 64 + 64]
                        if on_act and hi == 1:  # split muls across ACT/DVE
                            nc.scalar.mul(dst, t[:, jj, 0:64], r_ap)
                        else:
                            nc.vector.tensor_scalar(
                                out=dst, in0=t[:, jj, 0:64], scalar1=r_ap,
                                scalar2=None, op0=MULT)
                lo, hi_r = quads[0] * 128, (quads[-1] + 1) * 128
                nc.sync.dma_start(
                    out=out_o[lo:hi_r, p * 128:(p + 1) * 128].rearrange(
                        "(qc qi) c -> qi qc c", qi=128),
                    in_=ost[:, quads[0]:quads[-1] + 1, :])

            if last:
                norm_store(pv1, quads1, on_act=False)
                # qc 4..6 from prjL slots 1..6
                r2 = outp.tile([128, 6, 1], F32, tag="rA", name="r2")
                nc.vector.reciprocal(out=r2[:], in_=prjL[:, 1:7, 64:65])
                for hi in range(2):
                    for qi in range(3):
                        qc = 4 + qi
                        slot = 1 + hi * 3 + qi
                        dst = ost[:, qc, hi * 64:hi * 64 + 64]
                        if hi == 1:
                            nc.scalar.mul(dst, prjL[:, slot, 0:64],
                                          r2[:, slot - 1, :])
                        else:
                            nc.vector.tensor_scalar(
                                out=dst, in0=prjL[:, slot, 0:64],
                                scalar1=r2[:, slot - 1, :], scalar2=None,
                                op0=MULT)
                # qc 7: re-accumulate from the kept prob tiles in the pv bank
                pv3 = pvp.tile([128, 2, 73], F32, tag="pv", name="pv3")
                q7 = slice(7 * 128, 8 * 128)
                for kb in range(KB):
                    for hi in range(2):
                        nc.tensor.matmul(pv3[:, hi, 0:65],
                                         es_all[kb][hi][:, q7],
                                         vvs[:, kb, 2 * p + hi, 0:65],
                                         start=(kb == 0 and hi == 0),
                                         stop=False, skip_group_check=True)
                for hi in range(2):
                    nc.tensor.matmul(pv3[:, hi, 0:65], eq7[hi][:, q7],
                                     vvs[:, KB - 1, 2 * p + hi, 0:65],
                                     start=False, stop=True,
                                     skip_group_check=True)
                r3 = outp.tile([128, 2, 1], F32, tag="rB", name="r3")
                nc.vector.reciprocal(out=r3[:], in_=pv3[:, :, 64:65])
                for hi in range(2):
                    dst = ost[:, 7, hi * 64:hi * 64 + 64]
                    if hi == 1:
                        nc.scalar.mul(dst, pv3[:, hi, 0:64], r3[:, hi, :])
                    else:
                        nc.vector.tensor_scalar(
                            out=dst, in0=pv3[:, hi, 0:64],
                            scalar1=r3[:, hi, :], scalar2=None, op0=MULT)
                nc.sync.dma_start(
                    out=out_o[512:1024, p * 128:(p + 1) * 128].rearrange(
                        "(qc qi) c -> qi qc c", qi=128),
                    in_=ost[:, 4:8, :])
            else:
                norm_store(pv1, quads1, on_act=False)

                def gen_deferred(p=p, es_all=es_all, eq7=eq7, ost=ost):
                    for quads in ([2, 3], [4, 5], [6, 7]):
                        pvt = (pvp.tile([128, 4, 73], F32, tag="pv",
                                        name="pvd"), None)
                        for kb in range(KB):
                            emit_pv2(p, pvt, quads, kb, es_all[kb],
                                     start=(kb == 0), stop=False)
                            yield 120
                        emit_pv2(p, pvt, quads, KB - 1, eq7,
                                 start=False, stop=True)
                        yield 120
                        norm_store(pvt, quads, ost=ost, p=p, on_act=True)
                        yield 0
                bg.appendleft(gen_deferred())

    nc.finalize()
    return nc


def _get_compiled():
    global _compiled
    if _compiled is None:
        _compiled = _build()
    return _compiled


def kernel(x, query, Wkv, bkv, Wq, bq):
    import ml_dtypes
    from concourse.bass_utils import run_bass_kernel_spmd

    bf16 = ml_dtypes.bfloat16
    x = np.asarray(x, dtype=np.float32)
    query = np.asarray(query, dtype=np.float32)
    Wkv = np.asarray(Wkv, dtype=np.float32)
    bkv = np.asarray(bkv, dtype=np.float32)
    Wq = np.asarray(Wq, dtype=np.float32)
    bq = np.asarray(bq, dtype=np.float32)

    def tile_T(a):  # [N, C] -> [128, CT, N] (a.T tiled over contraction)
        return np.ascontiguousarray(
            a.T.reshape(CT, 128, N).transpose(1, 0, 2)).astype(bf16)

    def tile_w(w):  # [C, EC] -> [128, PAIRS, CT, 128]
        return np.ascontiguousarray(
            w.reshape(CT, 128, PAIRS, 128).transpose(1, 2, 0, 3)).astype(bf16)

    in_maps = []
    for core in range(NCORES):
        b, hg = core // 2, core % 2
        ecs = slice(hg * EC, (hg + 1) * EC)
        bv = bkv[E + hg * EC:E + (hg + 1) * EC]
        in_maps.append({
            "xT": tile_T(x[b]),
            "qryT": tile_T(query[b]),
            "wq": tile_w(Wq[:, ecs]),
            "wk": tile_w(Wkv[:, hg * EC:(hg + 1) * EC]),
            "wv": tile_w(Wkv[:, E + hg * EC:E + (hg + 1) * EC]),
            "bqc": np.ascontiguousarray(bq[ecs].reshape(PAIRS, 128).T),
            "bkc": np.ascontiguousarray(
                bkv[hg * EC:(hg + 1) * EC].reshape(PAIRS, 128).T),
            "bvv": np.ascontiguousarray(
                np.tile(bv[None, :], (128, 1)).astype(bf16)),
        })

    nc = _get_compiled()
    out = np.empty((B, N, E), np.float32)
    last_err = None
    for attempt in range(4):
        try:
            res = run_bass_kernel_spmd(nc, in_maps, list(range(NCORES)))
        except Exception as ex:  # transient NRT_EXEC_UNIT_UNRECOVERABLE etc.
            last_err = ex
            continue
        for core in range(NCORES):
            b, hg = core // 2, core % 2
            out[b, :, hg * EC:(hg + 1) * EC] = res.results[core]["out_t"]
        if np.isfinite(out).all():  # guard against transient device flakes
            return out
        last_err = RuntimeError("non-finite values in kernel output")
    raise last_err
